# revision 27
# baseline (speedup 1.0000x reference)
"""Trainium2 Bass kernel: batched 8x8-block IDCT (dequant + 2D separable transform).

Math per 8x8 block b of each 1024x1024 image:
    out_b = mtx.T @ (qtable * b) @ mtx + 128

Implementation (per core, pure data parallel over the batch dim):
  - Each of 8 cores gets 4 images = 4096 rows x 1024 cols, processed as 32
    slabs of 128 rows.
  - All device I/O is fp16 (tolerance is 2e-2; fp16 adds ~5e-4): the host
    casts x to fp16 before upload and the kernel writes an fp16 result that
    the host casts back, halving HBM traffic in both directions.
  - Per slab: dequantize elementwise with a pre-tiled qtable (DVE, fp16 2x
    mode), then for each 128x128 chunk two fp16 matmuls with the data as the
    *stationary* operand and C = kron(I_16, mtx) as the moving operand:
        P1_c = Xd_c^T @ C    (row-pass, output lands transposed: (w, i))
        P2_c = P1_c^T @ C    (col-pass, output back in (i, j) orientation)
    The PSUM->SBUF copy after pass 1 is column-split between DVE and ACT to
    balance engine load; the +128 rides as a bias on the final ACT copy.
  - Input DMAs are 4-slab (1 MiB) batches issued from the sync engine
    (qSPDynamicHW ring); output DMAs issue from the otherwise-idle GPSIMD
    engine (SWDGE ring) so neither compute engine pays the ~0.6us HWDGE
    issue cost and the two streams never head-of-line block each other.
  - Host side only shards/gathers, casts dtypes, and builds the two small
    constants.
"""

import numpy as np

_N_CORES = 8
_B, _H, _W = 32, 1024, 1024
_PER = _B // _N_CORES            # images per core
_ROWS = _PER * _H                # 4096 rows per core
_SLABS = _ROWS // 128            # 32 slabs of 128 rows
_INB = 4                         # slabs per input DMA batch

# Column split of the pass-1 PSUM->SBUF copy: DVE takes [0, _XS), the
# scalar engine takes [_XS, 1024) plus the whole final copy.
_XS = 640

_nc_cache = None


def _build_nc():
    from contextlib import ExitStack

    import concourse.bass as bass
    import concourse.tile as tile
    from concourse import mybir

    F16 = mybir.dt.float16
    F32 = mybir.dt.float32
    nc = bass.Bass()
    x_in = nc.declare_dram_parameter("x", [_SLABS, 128, _W], F16, isOutput=False)
    qt_in = nc.declare_dram_parameter("qtile", [128, _W], F16, isOutput=False)
    c_in = nc.declare_dram_parameter("cmat", [128, 128], F16, isOutput=False)
    y_out = nc.declare_dram_parameter("y", [_ROWS, _W], F16, isOutput=True)

    with ExitStack() as ctx:
        tc = ctx.enter_context(tile.TileContext(nc))
        const = ctx.enter_context(tc.tile_pool(name="const", bufs=1))
        xp = ctx.enter_context(tc.tile_pool(name="xp", bufs=3))
        xdp = ctx.enter_context(tc.tile_pool(name="xdp", bufs=6))
        s1p = ctx.enter_context(tc.tile_pool(name="s1p", bufs=6))
        op = ctx.enter_context(tc.tile_pool(name="op", bufs=6))
        p1p = ctx.enter_context(tc.tile_pool(name="p1p", bufs=2, space="PSUM"))
        p2p = ctx.enter_context(tc.tile_pool(name="p2p", bufs=2, space="PSUM"))

        # Slab 0's input goes on the wire first so compute can start as
        # early as possible; the small constants queue up behind it.
        xt = xp.tile([128, _W], F16)
        nc.sync.dma_start(
            xt[:].rearrange("p (j c) -> p j c", j=1),
            x_in[0:1].transpose([1, 0, 2]),
        )
        qt = const.tile([128, _W], F16)
        nc.sync.dma_start(qt[:], qt_in[:])
        cm = const.tile([128, 128], F16)
        nc.sync.dma_start(cm[:], c_in[:])
        bias128 = const.tile([128, 1], F32)
        nc.vector.memset(bias128[:], 128.0)

        # Touch the constants once so their DMA waits are absorbed here;
        # steady-state instructions then carry a single wait each (walrus
        # rejects instructions with too many sync waits).
        scratch = const.tile([128, 2], F16)
        nc.vector.tensor_copy(scratch[:], qt[:, :2])
        # Warm-up: ~4us of back-to-back dummy matmuls while the first input
        # DMA is in flight, so the PE_HAM clock gate opens (1.2 -> 2.4 GHz)
        # before the real matmuls start.
        p1 = p1p.tile([128, _W], F32)
        for _ in range(60):
            nc.tensor.matmul(p1[:, :128], cm[:], cm[:], start=True, stop=True)

        # Depth-2 software pipeline. At emission step s:
        #   stage A (slab s):   input DMA (1 MiB 4-slab batches), dequant,
        #                       pass-1 matmuls
        #   stage B (slab s-1): PSUM->SBUF copy of p1 (split DVE/ACT), pass-2
        #   stage C (slab s-2): +128 copy of p2 (ACT), output DMA (GPSIMD)
        # One-slab lag between producing a PSUM tile and consuming it keeps
        # most cross-engine semaphore waits pre-satisfied.
        batch_at = {1: 1, 2: 2, 4: 4}
        batch_at.update({s: _INB for s in range(8, _SLABS, _INB)})

        def emit_a(s):
            nonlocal xt, xoff
            if s in batch_at:
                nb = batch_at[s]
                xt = xp.tile([128, nb * _W], F16)
                nc.sync.dma_start(
                    xt[:].rearrange("p (j c) -> p j c", j=nb),
                    x_in[s : s + nb].transpose([1, 0, 2]),
                )
                xoff = 0
            xd = xdp.tile([128, _W], F16)
            nc.vector.tensor_mul(xd[:], xt[:, xoff : xoff + _W], qt[:])
            xoff += _W
            p1 = p1p.tile([128, _W], F32)
            for c in range(8):
                nc.tensor.matmul(
                    p1[:, 128 * c : 128 * (c + 1)],
                    xd[:, 128 * c : 128 * (c + 1)],
                    cm[:],
                    start=True,
                    stop=True,
                )
            return p1

        def emit_b(p1):
            s1 = s1p.tile([128, _W], F16)
            nc.vector.tensor_copy(s1[:, :_XS], p1[:, :_XS])
            nc.scalar.copy(s1[:, _XS:], p1[:, _XS:])
            p2 = p2p.tile([128, _W], F32)
            for c in range(8):
                nc.tensor.matmul(
                    p2[:, 128 * c : 128 * (c + 1)],
                    s1[:, 128 * c : 128 * (c + 1)],
                    cm[:],
                    start=True,
                    stop=True,
                )
            return p2

        def emit_c(s, p2):
            ot = op.tile([128, _W], F16)
            nc.scalar.activation(
                ot[:], p2[:], mybir.ActivationFunctionType.Identity, bias=bias128[:]
            )
            nc.gpsimd.dma_start(y_out[128 * s : 128 * (s + 1), :], ot[:])

        xoff = 0
        stage = {}
        for s in range(_SLABS + 2):
            if s < _SLABS:
                stage[s] = emit_a(s)
            if 1 <= s <= _SLABS:
                stage[s - 1] = emit_b(stage[s - 1])
            if s >= 2:
                emit_c(s - 2, stage.pop(s - 2))

    _split_excess_waits(nc, mybir)
    return nc


def _split_excess_waits(nc, mybir):
    """Walrus allows a limited number of sync waits per lowered instruction
    (1 for DMA/DVE/ACT structs, a couple for matmul via the LDWEIGHTS pair,
    2 per EventSemaphore). Tile's wait assignment can attach more; move the
    excess onto standalone same-engine EventSemaphore carriers."""

    def budget(inst):
        tn = type(inst).__name__
        if tn == "InstEventSemaphore":
            return 2
        return 1

    wid = 0
    for fn in nc.m.functions:
        for bb in fn.blocks:
            out = []
            for inst in bb.instructions:
                si = inst.sync_info
                waits = list(si.on_wait) if si is not None else []
                b = budget(inst)
                if len(waits) > b:
                    extra, keep = waits[:-b], waits[-b:]
                    for i in range(0, len(extra), 2):
                        ev = mybir.InstEventSemaphore(
                            name=f"WSPLIT-{wid}", ins=[], outs=[]
                        )
                        wid += 1
                        ev.engine = inst.engine
                        ev.sync_info = mybir.SyncInfo(
                            on_wait=extra[i : i + 2], on_update=[]
                        )
                        out.append(ev)
                    inst.sync_info = mybir.SyncInfo(
                        on_wait=keep, on_update=list(si.on_update)
                    )
                out.append(inst)
            bb.instructions = out


def _get_nc():
    global _nc_cache
    if _nc_cache is None:
        _nc_cache = _build_nc()
    return _nc_cache


def _run(x, qtable, mtx, trace=False, **kwargs):
    from concourse.bass_utils import run_bass_kernel_spmd

    x = np.asarray(x, dtype=np.float32).reshape(_B * _H, _W)
    x16 = np.ascontiguousarray(x.astype(np.float16))
    qtable = np.asarray(qtable, dtype=np.float32)
    mtx = np.asarray(mtx, dtype=np.float32)
    qtile = np.ascontiguousarray(
        np.tile(qtable, (16, _W // 8)).astype(np.float16)
    )
    cmat = np.ascontiguousarray(
        np.kron(np.eye(16, dtype=np.float32), mtx).astype(np.float16)
    )

    in_maps = [
        {
            "x": np.ascontiguousarray(x16[i * _ROWS : (i + 1) * _ROWS]).reshape(
                _SLABS, 128, _W
            ),
            "qtile": qtile,
            "cmat": cmat,
        }
        for i in range(_N_CORES)
    ]
    res = run_bass_kernel_spmd(
        _get_nc(), in_maps, list(range(_N_CORES)), trace=trace, **kwargs
    )
    out = np.concatenate([res.results[i]["y"] for i in range(_N_CORES)], axis=0)
    return (
        out.astype(np.float32).reshape(_B, 1, _H, _W),
        res,
    )


def kernel(x, qtable, mtx):
    out, _ = _run(x, qtable, mtx, trace=False)
    return out


# revision 28
# speedup vs baseline: 1.1479x; 1.1479x over previous
"""Trainium2 Bass kernel: batched 8x8-block IDCT (dequant + 2D separable transform).

Math per 8x8 block b of each 1024x1024 image:
    out_b = mtx.T @ (qtable * b) @ mtx + 128

Implementation (per core, pure data parallel over the batch dim):
  - Each of 8 cores gets 4 images = 4096 rows x 1024 cols, processed as 32
    slabs of 128 rows.
  - All device I/O is fp16 (tolerance is 2e-2; fp16 adds ~5e-4): the host
    casts x to fp16 before upload and the kernel writes an fp16 result that
    the host casts back, halving HBM traffic in both directions.
  - Per slab: dequantize elementwise with a pre-tiled qtable (DVE, fp16 2x
    mode), then for each 128x128 chunk two fp16 matmuls with the data as the
    *stationary* operand and C = kron(I_16, mtx) as the moving operand:
        P1_c = Xd_c^T @ C    (row-pass, output lands transposed: (w, i))
        P2_c = P1_c^T @ C    (col-pass, output back in (i, j) orientation)
    The PSUM->SBUF copy after pass 1 is column-split between DVE and ACT to
    balance engine load; the +128 rides as a bias on the final ACT copy.
  - Input DMAs are 4-slab (1 MiB) batches issued from the sync engine
    (qSPDynamicHW ring); output DMAs issue from the otherwise-idle GPSIMD
    engine (SWDGE ring) so neither compute engine pays the ~0.6us HWDGE
    issue cost and the two streams never head-of-line block each other.
  - Host side only shards/gathers, casts dtypes, and builds the two small
    constants.
"""

import numpy as np

_N_CORES = 8
_B, _H, _W = 32, 1024, 1024
_PER = _B // _N_CORES            # images per core
_ROWS = _PER * _H                # 4096 rows per core
_SLABS = _ROWS // 128            # 32 slabs of 128 rows
_INB = 4                         # slabs per input DMA batch

# Column split of the pass-1 PSUM->SBUF copy: DVE takes [0, _XS), the
# scalar engine takes [_XS, 1024) plus the whole final copy.
_XS = 640

_nc_cache = None


def _build_nc():
    from contextlib import ExitStack

    import concourse.bass as bass
    import concourse.tile as tile
    from concourse import mybir

    F16 = mybir.dt.float16
    F32 = mybir.dt.float32
    nc = bass.Bass()
    x_in = nc.declare_dram_parameter("x", [_SLABS, 128, _W], F16, isOutput=False)
    qt_in = nc.declare_dram_parameter("qtile", [128, _W], F16, isOutput=False)
    c_in = nc.declare_dram_parameter("cmat", [128, 128], F16, isOutput=False)
    y_out = nc.declare_dram_parameter("y", [_ROWS, _W], F16, isOutput=True)

    with ExitStack() as ctx:
        tc = ctx.enter_context(tile.TileContext(nc))
        const = ctx.enter_context(tc.tile_pool(name="const", bufs=1))
        xp = ctx.enter_context(tc.tile_pool(name="xp", bufs=3))
        xdp = ctx.enter_context(tc.tile_pool(name="xdp", bufs=6))
        s1p = ctx.enter_context(tc.tile_pool(name="s1p", bufs=6))
        op = ctx.enter_context(tc.tile_pool(name="op", bufs=6))
        p1p = ctx.enter_context(tc.tile_pool(name="p1p", bufs=2, space="PSUM"))
        p2p = ctx.enter_context(tc.tile_pool(name="p2p", bufs=2, space="PSUM"))

        # The first input batch goes on the wire before the constants so
        # compute can start as early as possible; the small constants queue
        # up behind it and still land in time for the first dequant.
        xt = xp.tile([128, _INB * _W], F16)
        nc.sync.dma_start(
            xt[:].rearrange("p (j c) -> p j c", j=_INB),
            x_in[0:_INB].transpose([1, 0, 2]),
        )
        qt = const.tile([128, _W], F16)
        nc.sync.dma_start(qt[:], qt_in[:])
        cm = const.tile([128, 128], F16)
        nc.sync.dma_start(cm[:], c_in[:])
        bias128 = const.tile([128, 1], F32)
        nc.vector.memset(bias128[:], 128.0)

        # Touch the constants once so their DMA waits are absorbed here;
        # steady-state instructions then carry a single wait each (walrus
        # rejects instructions with too many sync waits).
        scratch = const.tile([128, 2], F16)
        nc.vector.tensor_copy(scratch[:], qt[:, :2])
        # Warm-up: ~4us of back-to-back dummy matmuls while the first input
        # DMA is in flight, so the PE_HAM clock gate opens (1.2 -> 2.4 GHz)
        # before the real matmuls start.
        p1 = p1p.tile([128, _W], F32)
        for _ in range(60):
            nc.tensor.matmul(p1[:, :128], cm[:], cm[:], start=True, stop=True)

        # Depth-2 software pipeline. At emission step s:
        #   stage A (slab s):   input DMA (1 MiB 4-slab batches), dequant,
        #                       pass-1 matmuls
        #   stage B (slab s-1): PSUM->SBUF copy of p1 (split DVE/ACT), pass-2
        #   stage C (slab s-2): +128 copy of p2 (ACT), output DMA (GPSIMD)
        # One-slab lag between producing a PSUM tile and consuming it keeps
        # most cross-engine semaphore waits pre-satisfied.
        batch_at = {s: _INB for s in range(_INB, _SLABS, _INB)}

        def emit_a(s):
            nonlocal xt, xoff
            if s in batch_at:
                nb = batch_at[s]
                xt = xp.tile([128, nb * _W], F16)
                nc.sync.dma_start(
                    xt[:].rearrange("p (j c) -> p j c", j=nb),
                    x_in[s : s + nb].transpose([1, 0, 2]),
                )
                xoff = 0
            xd = xdp.tile([128, _W], F16)
            nc.vector.tensor_mul(xd[:], xt[:, xoff : xoff + _W], qt[:])
            xoff += _W
            p1 = p1p.tile([128, _W], F32)
            for c in range(8):
                nc.tensor.matmul(
                    p1[:, 128 * c : 128 * (c + 1)],
                    xd[:, 128 * c : 128 * (c + 1)],
                    cm[:],
                    start=True,
                    stop=True,
                )
            return p1

        def emit_b(p1):
            s1 = s1p.tile([128, _W], F16)
            nc.vector.tensor_copy(s1[:, :_XS], p1[:, :_XS])
            nc.scalar.copy(s1[:, _XS:], p1[:, _XS:])
            p2 = p2p.tile([128, _W], F32)
            for c in range(8):
                nc.tensor.matmul(
                    p2[:, 128 * c : 128 * (c + 1)],
                    s1[:, 128 * c : 128 * (c + 1)],
                    cm[:],
                    start=True,
                    stop=True,
                )
            return p2

        def emit_c(s, p2):
            ot = op.tile([128, _W], F16)
            nc.scalar.activation(
                ot[:], p2[:], mybir.ActivationFunctionType.Identity, bias=bias128[:]
            )
            nc.gpsimd.dma_start(y_out[128 * s : 128 * (s + 1), :], ot[:])

        xoff = 0
        stage = {}
        for s in range(_SLABS + 2):
            if s < _SLABS:
                stage[s] = emit_a(s)
            if 1 <= s <= _SLABS:
                stage[s - 1] = emit_b(stage[s - 1])
            if s >= 2:
                emit_c(s - 2, stage.pop(s - 2))

    _split_excess_waits(nc, mybir)
    return nc


def _split_excess_waits(nc, mybir):
    """Walrus allows a limited number of sync waits per lowered instruction
    (1 for DMA/DVE/ACT structs, a couple for matmul via the LDWEIGHTS pair,
    2 per EventSemaphore). Tile's wait assignment can attach more; move the
    excess onto standalone same-engine EventSemaphore carriers."""

    def budget(inst):
        tn = type(inst).__name__
        if tn == "InstEventSemaphore":
            return 2
        return 1

    wid = 0
    for fn in nc.m.functions:
        for bb in fn.blocks:
            out = []
            for inst in bb.instructions:
                si = inst.sync_info
                waits = list(si.on_wait) if si is not None else []
                b = budget(inst)
                if len(waits) > b:
                    extra, keep = waits[:-b], waits[-b:]
                    for i in range(0, len(extra), 2):
                        ev = mybir.InstEventSemaphore(
                            name=f"WSPLIT-{wid}", ins=[], outs=[]
                        )
                        wid += 1
                        ev.engine = inst.engine
                        ev.sync_info = mybir.SyncInfo(
                            on_wait=extra[i : i + 2], on_update=[]
                        )
                        out.append(ev)
                    inst.sync_info = mybir.SyncInfo(
                        on_wait=keep, on_update=list(si.on_update)
                    )
                out.append(inst)
            bb.instructions = out


def _get_nc():
    global _nc_cache
    if _nc_cache is None:
        _nc_cache = _build_nc()
    return _nc_cache


def _run(x, qtable, mtx, trace=False, **kwargs):
    from concourse.bass_utils import run_bass_kernel_spmd

    x = np.asarray(x, dtype=np.float32).reshape(_B * _H, _W)
    x16 = np.ascontiguousarray(x.astype(np.float16))
    qtable = np.asarray(qtable, dtype=np.float32)
    mtx = np.asarray(mtx, dtype=np.float32)
    qtile = np.ascontiguousarray(
        np.tile(qtable, (16, _W // 8)).astype(np.float16)
    )
    cmat = np.ascontiguousarray(
        np.kron(np.eye(16, dtype=np.float32), mtx).astype(np.float16)
    )

    in_maps = [
        {
            "x": np.ascontiguousarray(x16[i * _ROWS : (i + 1) * _ROWS]).reshape(
                _SLABS, 128, _W
            ),
            "qtile": qtile,
            "cmat": cmat,
        }
        for i in range(_N_CORES)
    ]
    res = run_bass_kernel_spmd(
        _get_nc(), in_maps, list(range(_N_CORES)), trace=trace, **kwargs
    )
    out = np.concatenate([res.results[i]["y"] for i in range(_N_CORES)], axis=0)
    return (
        out.astype(np.float32).reshape(_B, 1, _H, _W),
        res,
    )


def kernel(x, qtable, mtx):
    out, _ = _run(x, qtable, mtx, trace=False)
    return out
